# revision 34
# baseline (speedup 1.0000x reference)
"""AtomGMMProjector Bass kernel for Trainium2 (8 NeuronCores, SPMD).

Math (per batch b):
    cx = centers @ R[b,0], cy = centers @ R[b,1]          (rotated atom x/y)
    z_{x,y}[n,d] = (line[d] - c[n]) / (sqrt(2)*sigma[n])  (scaled distance)
    proj[y,x]    = sum_n amp[n] * exp(-zy[n,y]^2) * exp(-zx[n,x]^2)

Kernel layout: atoms on SBUF partitions (atom n -> partition n//32, column
n%32), line coords along the free axis.  Per 128-atom tile:
    z  = tensor_scalar(line_b, mul s'[n], add -c[n]*s'[n])   (DVE, 2x fp32)
    e  = Derivative_Erf(z) = 2/sqrt(pi)*exp(-z^2)            (ACT, batched
         across G tiles per op; the (2/sqrt(pi))^2 factor is undone by a
         PI/4 scale on the PSUM->SBUF output copy)
    ax = e_x * amp[n]                                        (DVE, 4x bf16)
    out += e_y.T @ ax  via accumulating bf16 PE matmuls (fp32 PSUM)

Sharding: data-parallel over batch B=32 -> 4 batches per core, 8 cores.
"""

import numpy as np
from contextlib import ExitStack

import concourse.bass as bass
import concourse.bacc as bacc
import concourse.mybir as mybir
import concourse.tile as tile
from concourse.bass_utils import run_bass_kernel_spmd

F32 = mybir.dt.float32
F32R = mybir.dt.float32r
BF16 = mybir.dt.bfloat16
AF = mybir.ActivationFunctionType
OP = mybir.AluOpType

B, N, D = 32, 4096, 256
NCORES = 8
BPC = B // NCORES          # batches per core
NT = N // 128              # atom tiles (atom n -> partition n//32, col n%32)
G = 16                     # tiles per exp/square group
INV_SQRT2 = 0.7071067811865476

Z_DT = BF16                # z and z^2 buffers (bf16 -> 2x DVE modes)
E_DT = BF16                # exp/ax buffers; bf16 matmuls lower to LDW+MM,
                           # which splits sync waits (f32r allows only one)
AMP_ON_GPSIMD = True       # amp-weighting op placement
ZY_ON_GPSIMD = False       # y-axis z-build op placement
USE_ERFD = True            # Derivative_Erf = (2/sqrt(pi))exp(-x^2) fuses
                           # square+exp into one ACT op (table-supported on
                           # trn2); False falls back to TT square + Exp
SKIP = set()               # bench bisection: subset of {z,sq,exp,amp,mm,out}


def _build_kernel(repeats=1):
    nc = bacc.Bacc("TRN2", target_bir_lowering=False, debug=False)
    line_d = nc.dram_tensor("line_coords", [D], F32, kind="ExternalInput")
    rot_d = nc.dram_tensor("rot_mats", [BPC, 3, 3], F32, kind="ExternalInput")
    cen_d = nc.dram_tensor("centers", [BPC, N, 3], F32, kind="ExternalInput")
    sig_d = nc.dram_tensor("sigmas", [N], F32, kind="ExternalInput")
    amp_d = nc.dram_tensor("amplitudes", [N], F32, kind="ExternalInput")
    out_d = nc.dram_tensor("out", [BPC, D, D], F32, kind="ExternalOutput")

    with tile.TileContext(nc) as tc, ExitStack() as ctx:
        pools = _make_pools(ctx, tc)
        consts = _setup(pools, tc, line_d.ap(), rot_d.ap(), sig_d.ap(),
                        amp_d.ap())
        for _ in range(repeats):
            _body(pools, consts, tc, cen_d.ap(), out_d.ap())
    nc.compile()
    return nc


def _make_pools(ctx, tc):
    return {
        "const": ctx.enter_context(tc.tile_pool(name="const", bufs=1)),
        "cen": ctx.enter_context(tc.tile_pool(name="cen", bufs=2)),
        "coef": ctx.enter_context(tc.tile_pool(name="coef", bufs=2)),
        "z": ctx.enter_context(tc.tile_pool(name="z", bufs=3)),
        "sq": ctx.enter_context(tc.tile_pool(name="sq", bufs=2)),
        "e": ctx.enter_context(tc.tile_pool(name="e", bufs=3)),
        "ax": ctx.enter_context(tc.tile_pool(name="ax", bufs=8)),
        "osb": ctx.enter_context(tc.tile_pool(name="osb", bufs=2)),
        "ps": ctx.enter_context(tc.tile_pool(name="ps", bufs=2, space="PSUM")),
        "bcps": ctx.enter_context(tc.tile_pool(name="bcps", bufs=1, space="PSUM")),
    }


def _setup(pools, tc, line, rot, sig, amp):
    nc = tc.nc
    const = pools["const"]
    bcps = pools["bcps"]

    # ---- broadcast line coords + rotation entries to all 128 partitions ----
    # staging row [1, D + 9*BPC] -> ones-matmul -> PSUM [128, .] -> SBUF
    nstg = D + 9 * BPC
    stg = const.tile([1, nstg], F32)
    nc.sync.dma_start(stg[0:1, 0:D], line[None, :])
    nc.sync.dma_start(stg[0:1, D:nstg], rot.rearrange("b i j -> (b i j)")[None, :])
    ones = const.tile([1, 128], F32)
    nc.vector.memset(ones[:], 1.0)
    # route stg through a DVE copy so the (self-loading fp32) broadcast
    # matmul depends on DVE only -- fp32 matmuls support a single sync wait
    stg2 = const.tile([1, nstg], F32)
    nc.vector.tensor_copy(stg2[:], stg[:])
    bc_ps = bcps.tile([128, nstg], F32)
    nc.tensor.matmul(bc_ps[:], lhsT=ones[:], rhs=stg2[:], start=True, stop=True)
    bc = const.tile([128, nstg], F32)
    nc.scalar.copy(bc[:], bc_ps[:])

    # ---- per-atom constants: s' = 1/(sqrt2*sigma), -s', amplitudes --------
    sig_t = const.tile([128, NT], F32)
    nc.sync.dma_start(sig_t[:], sig.rearrange("(p w) -> p w", p=128))
    amp_t = const.tile([128, NT], F32)
    nc.sync.dma_start(amp_t[:], amp.rearrange("(p w) -> p w", p=128))
    rec = const.tile([128, NT], F32)
    nc.vector.reciprocal(rec[:], sig_t[:])
    spos = const.tile([128, NT], F32)
    nc.vector.tensor_scalar(out=spos[:], in0=rec[:], scalar1=INV_SQRT2,
                            scalar2=None, op0=OP.mult)
    sneg = const.tile([128, NT], F32)
    nc.vector.tensor_scalar(out=sneg[:], in0=rec[:], scalar1=-INV_SQRT2,
                            scalar2=None, op0=OP.mult)
    return {"bc": bc, "amp_t": amp_t, "spos": spos, "sneg": sneg}


def _body(pools, consts, tc, cen, out):
    nc = tc.nc
    cenp = pools["cen"]
    coefp = pools["coef"]
    zp = pools["z"]
    sqp = pools["sq"]
    ep = pools["e"]
    axp = pools["ax"]
    outp = pools["osb"]
    psp = pools["ps"]
    bc = consts["bc"]
    amp_t = consts["amp_t"]
    spos = consts["spos"]
    sneg = consts["sneg"]
    line_b = bc[:, 0:D]

    def rsc(b, i, j):  # R[b,i,j] broadcast column [128,1]
        o = D + 9 * b + 3 * i + j
        return bc[:, o:o + 1]

    for b in range(BPC):
        # ---- load centers; rotate into image frame; bias = -c*s' ---------
        cen_t = cenp.tile([128, 3 * NT], F32)
        nc.sync.dma_start(cen_t[:], cen[b].rearrange("(p w) c -> p (w c)", p=128))
        cen_v = cen_t[:].rearrange("p (w c) -> p w c", c=3)
        bias = []
        for i in range(2):  # i=0 -> x axis, i=1 -> y axis
            t0 = coefp.tile([128, NT], F32, tag="t0")
            nc.vector.tensor_scalar(out=t0[:], in0=cen_v[:, :, 0],
                                    scalar1=rsc(b, i, 0), scalar2=None, op0=OP.mult)
            nc.vector.scalar_tensor_tensor(out=t0[:], in0=cen_v[:, :, 1],
                                           scalar=rsc(b, i, 1), in1=t0[:],
                                           op0=OP.mult, op1=OP.add)
            nc.vector.scalar_tensor_tensor(out=t0[:], in0=cen_v[:, :, 2],
                                           scalar=rsc(b, i, 2), in1=t0[:],
                                           op0=OP.mult, op1=OP.add)
            bn = coefp.tile([128, NT], F32, tag=f"bn{i}")
            nc.vector.tensor_tensor(out=bn[:], in0=t0[:], in1=sneg[:], op=OP.mult)
            bias.append(bn)
        bnx, bny = bias

        ps = [psp.tile([128, D], F32, tag=f"ps{h}", name=f"ps{h}") for h in range(2)]
        for g in range(NT // G):
            zb = zp.tile([128, 2 * D * G], Z_DT)
            if "z" not in SKIP:
                zy_eng = nc.gpsimd if ZY_ON_GPSIMD else nc.vector
                for l in range(G):
                    a = g * G + l
                    nc.vector.tensor_scalar(
                        out=zb[:, 2 * D * l:2 * D * l + D], in0=line_b,
                        scalar1=spos[:, a:a + 1], scalar2=bnx[:, a:a + 1],
                        op0=OP.mult, op1=OP.add)
                    zy_eng.tensor_scalar(
                        out=zb[:, 2 * D * l + D:2 * D * (l + 1)], in0=line_b,
                        scalar1=spos[:, a:a + 1], scalar2=bny[:, a:a + 1],
                        op0=OP.mult, op1=OP.add)
            # erf'(z) = (2/sqrt(pi)) * exp(-z^2): square+exp in ONE ACT op.
            # The (2/sqrt(pi))^2 factor is undone by PI/4 in the output copy.
            eb = ep.tile([128, 2 * D * G], E_DT)
            if "exp" not in SKIP:
                if USE_ERFD:
                    nc.scalar.activation(out=eb[:], in_=zb[:],
                                         func=AF.Derivative_Erf)
                else:
                    sq = sqp.tile([128, 2 * D * G], Z_DT)
                    nc.vector.tensor_tensor(out=sq[:], in0=zb[:], in1=zb[:],
                                            op=OP.mult)
                    nc.scalar.activation(out=eb[:], in_=sq[:], func=AF.Exp,
                                         scale=-1.0)
            for l in range(G):
                a = g * G + l
                ex = eb[:, 2 * D * l:2 * D * l + D]
                ey = eb[:, 2 * D * l + D:2 * D * (l + 1)]
                ax = axp.tile([128, D], E_DT)
                if "amp" not in SKIP:
                    eng = nc.gpsimd if AMP_ON_GPSIMD else nc.vector
                    eng.tensor_scalar(out=ax[:], in0=ex,
                                      scalar1=amp_t[:, a:a + 1],
                                      scalar2=None, op0=OP.mult)
                if "mm" not in SKIP:
                    for h in range(2):
                        nc.tensor.matmul(
                            ps[h][:], lhsT=ey[:, 128 * h:128 * (h + 1)],
                            rhs=ax[:],
                            start=(a == 0), stop=(a == NT - 1))
        if "out" not in SKIP:
            osb = outp.tile([128, 2 * D], F32)
            for h in range(2):
                # PI/4 undoes the two 2/sqrt(pi) factors from Derivative_Erf
                oscale = 0.7853981633974483 if USE_ERFD else 1.0
                nc.scalar.activation(out=osb[:, D * h:D * (h + 1)],
                                     in_=ps[h][:], func=AF.Copy,
                                     scale=oscale)
                nc.sync.dma_start(out[b, 128 * h:128 * (h + 1), :],
                                  osb[:, D * h:D * (h + 1)])


_NC_CACHE = {}


def _get_nc():
    if "nc" not in _NC_CACHE:
        _NC_CACHE["nc"] = _build_kernel()
    return _NC_CACHE["nc"]


def kernel(line_coords, rot_mats, centers, sigmas, amplitudes):
    line_coords = np.ascontiguousarray(np.asarray(line_coords, np.float32))
    rot_mats = np.ascontiguousarray(np.asarray(rot_mats, np.float32))
    centers = np.ascontiguousarray(np.asarray(centers, np.float32))
    sigmas = np.ascontiguousarray(np.asarray(sigmas, np.float32))
    amplitudes = np.ascontiguousarray(np.asarray(amplitudes, np.float32))

    nc = _get_nc()
    in_maps = []
    for c in range(NCORES):
        s = slice(c * BPC, (c + 1) * BPC)
        in_maps.append({
            "line_coords": line_coords,
            "rot_mats": np.ascontiguousarray(rot_mats[s]),
            "centers": np.ascontiguousarray(centers[s]),
            "sigmas": sigmas,
            "amplitudes": amplitudes,
        })
    res = run_bass_kernel_spmd(nc, in_maps, list(range(NCORES)))
    return np.concatenate([res.results[c]["out"] for c in range(NCORES)], axis=0)
